# revision 24
# baseline (speedup 1.0000x reference)
"""Trainium2 Bass kernel for GQA attention block (nn_Attention_36627481101235).

Reference computation (BS=1, SEQ=2048, DIM=4096, 32 q-heads, 8 kv-heads,
head_dim=128):
    q/k/v projections -> interleaved RoPE on q,k -> repeat_kv -> causal
    softmax attention -> output projection.

Sharding: tensor-parallel by heads over 8 cores. Core c gets q-heads
4c..4c+3 and kv-head c (GQA groups stay intact). Each core computes its
partial out = attn_out_c @ wo_c; the host sums the 8 bf16 partials in
f32. All matmul operands are bf16 (1 PE cycle/column, same as f32r, at
half the DMA traffic); psum accumulation is f32 throughout, RoPE tables
stay f32.

Design notes (all calibrated against TimelineSim, which matched the HW
number exactly on this harness):
  * PE cost = moving-operand columns x 0.4167ns, independent of output
    rows; Ldweights is free. PE work here is 331.9us: projections 164,
    scores+PV 58 (causal tiles sliced to live columns), out-proj 109.
  * The tensor engine has a p-state ramp: after ANY idle gap it runs
    at half clock until 3us of continuous execution. Many small stalls
    are worse than one big one, so the whole kernel is structured to
    keep one dense PE instruction stream.
  * The softmax denominator never touches the PE: exp'd score tiles
    (P_all, bf16) are pair-summed on DVE (2-byte 2x mode) with the
    first half of each head's pairs on the idle Pool engine, folded by
    a short tree, then one Pool partition_all_reduce broadcasts the
    per-q sum; reciprocal_approx_fast + one DVE multiply normalize ao
    during the psum evacuation. (The old ones-vector dcol matmul +
    K=1 broadcast matmul cost 32us of PE.)
  * Causal masking is multiplicative-post-exp: a 0/1 tril [128,128]
    multiply on the Pool engine (SBUF-only, so no Pool-PSUM access
    questions), applied to the diagonal block; dead slices of diagonal
    tiles are Pool-memset to zero so full-width tree adds stay exact.
  * Phase interleave: chunk sc's attention (B) is emitted as a Python
    generator woven between the k-batches of chunk sc+1's projection
    (A), so B's exp/DVE/Pool latency chains hide under dense
    projection matmuls. To free psum banks for this, A runs in two
    segments: q-heads first (4 ps6 banks), then k/v (reusing the
    slots the q RoPE evacs free), with x re-streamed from HBM for the
    second segment (DMA has ~55% headroom). B's score psums use a
    2-bank pool (sB); B's ao accumulators use 2 pacc banks so head
    finalize chains never gate the next chunk's projections.
  * B(0) (tiny, all-diagonal) weaves into A(1); B(3) weaves into the
    out-projection phase C between s-tile groups, with C(10) emitted
    mid-head as cover for B(3)h3's producer. C's psum evacuations run
    on DVE (ACT carries the exps). RoPE evacuation is 4 pure-DVE ops
    (cross-partition psum reads are legal when one operand is PSUM).
  * DMA: TimelineSim serializes all queues into one 360 B/ns
    resource, so ordering matters more than queue choice. Weights
    stream in batched k-slices (kb0 split k-tile-0-first for a ~3.2us
    time-to-first-matmul); kb0's k/v tiles are deferred to the end of
    A(0) (start flag on k-tile 4) to shrink the critical first batch;
    rope tables load per-chunk just in time (0.5MB slices); wo loads
    by 512-column blocks; the final s-tile drains in eighths with the
    last evac+DMA split in halves to shorten the tail chain.

Numerics: bf16 inputs/weights add ~0.4-0.5% each to the relative
error; bf16 pair-sum denominator ~0.3%; measured end-to-end vs the
fp32 reference: rel 7.1e-3 (budget 2e-2), HW-validated.

TimelineSim: 346.8us (PE busy ~95.7%, = the exact column-count floor
331.9us + ~5.5us startup DMA latency + ~4.3us drain tail + ~5us
residual micro-stalls). Baseline at session start was 427.7us.
Critical-path note: from ~45us on the PE stream is 97-100% dense; the
few remaining A(0)-region stalls are slack-absorbed (fixing them does
not move the total), and the tail is fixed DMA latency (issue 650 +
HWDGE 625 + DGE 650 + sem 900 + drain cascade). Further gains require
either cutting PE columns (blocked by the 2e-2 error budget ruling
out fp8) or breaking those fixed latencies.

WARNING (HW-verified failure mode): producing v pre-transposed by
making the x tile the matmul stationary with FOUR independent
accumulation regions (one per 128-col s-block) inside ONE psum bank
simulates fine but returns garbage on hardware (rel err 0.17) — do
not interleave multiple accumulation groups in a single psum bank.
The v path therefore keeps the explicit PE transpose via the
identity matrix. Other knobs that measured neutral-or-worse in sim:
p-state warmup matmuls, finer B_gen yields, deeper pall/scr/qTc/out
pools, half-width final po accumulation, shrinking final drain
pieces, ACT-queue tail DMAs, per-k-tile (vs pair) startup transfers.
fp8 (2x PE via DoubleRow) rejected: ~2% error would blow the 2e-2
budget.
"""
import numpy as np

import concourse.mybir as mybir
import concourse.tile as tile
from concourse import bacc
from concourse import bass_isa

BS, SEQ, DIM = 1, 2048, 4096
NH, DH = 4, 128          # q-heads per core, head dim
DQ = NH * DH             # 512
NCORES = 8
P = 128                  # partitions
SC = 512                 # s-chunk width
NSC = SEQ // SC          # 4
NKT = DIM // P           # 32 contraction tiles for projections
F32R = mybir.dt.float32r
F32 = mybir.dt.float32
BF16 = mybir.dt.bfloat16
NEG = -1e9


def build_nc(num_devices=NCORES):
    nc = bacc.Bacc("TRN2", target_bir_lowering=False, debug=False,
                   enable_asserts=False, num_devices=num_devices)
    xT = nc.dram_tensor("xT", (DIM, SEQ), BF16, kind="ExternalInput").ap()
    # host-packed startup stream: (wq k-tile | x k-tile) pairs for k=0..7
    # in exact consumption order, so kb0/kb1 stream as eight independent
    # 256KB pieces with per-piece semaphores (batch-level sems stall the PE
    # ~1.6us behind the bus; pair-level sems track it)
    boot = nc.dram_tensor("boot", (P, 8 * 2 * SC), BF16,
                          kind="ExternalInput").ap()
    wq = nc.dram_tensor("wq", (DIM, DQ), BF16, kind="ExternalInput").ap()
    wk = nc.dram_tensor("wk", (DIM, DH), BF16, kind="ExternalInput").ap()
    wv = nc.dram_tensor("wv", (DIM, DH), BF16, kind="ExternalInput").ap()
    wo = nc.dram_tensor("wo", (DQ, DIM), BF16, kind="ExternalInput").ap()
    ropeA = nc.dram_tensor("ropeA", (P, SEQ), F32R, kind="ExternalInput").ap()
    ropeB = nc.dram_tensor("ropeB", (P, SEQ), F32R, kind="ExternalInput").ap()
    masks = nc.dram_tensor("masks", (P, P), BF16, kind="ExternalInput").ap()
    ident = nc.dram_tensor("ident", (P, P), BF16, kind="ExternalInput").ap()
    out = nc.dram_tensor("out", (SEQ, DIM), BF16, kind="ExternalOutput").ap()

    with tile.TileContext(nc) as tc:
        with tc.tile_pool(name="persist", bufs=1) as pp, \
             tc.tile_pool(name="ps6", bufs=4, space="PSUM") as ps6, \
             tc.tile_pool(name="pacc", bufs=2, space="PSUM") as pacc, \
             tc.tile_pool(name="sB", bufs=2, space="PSUM") as sB:
            kT_sb = pp.tile([P, SEQ], BF16)             # rotated K^T [d, s]
            v_sb = pp.tile([P, SEQ], BF16)              # v tiles [s%128, st*128+d]
            aoT_sb = pp.tile([P, NH * SEQ], BF16)       # attn_outT [d, h*SEQ+s]
            ident_sb = pp.tile([P, P], BF16)

            from contextlib import ExitStack
            with tc.tile_pool(name="tab_p", bufs=1) as tab_p, \
                 tc.tile_pool(name="qTc_p", bufs=4) as qTc_p, \
                 tc.tile_pool(name="tmp_p", bufs=2) as tmp_p, \
                 tc.tile_pool(name="pall_p", bufs=2) as pall_p, \
                 tc.tile_pool(name="scr_p", bufs=2) as scr_p, \
                 tc.tile_pool(name="dsum_p", bufs=2) as dsum_p:
                inner = ExitStack()
                wq_p = inner.enter_context(tc.tile_pool(name="wq_p", bufs=1))
                wkv_p = inner.enter_context(tc.tile_pool(name="wkv_p", bufs=1))
                xt_p = inner.enter_context(tc.tile_pool(name="xt_p", bufs=6))
                boot_p = inner.enter_context(
                    tc.tile_pool(name="boot_p", bufs=1))
                vt_p = inner.enter_context(tc.tile_pool(name="vt_p", bufs=2))
                # weights, k-tile-major columns: col = k*width + local
                wq_sb = wq_p.tile([P, NKT * DQ], BF16)
                wk_sb = wkv_p.tile([P, NKT * DH], BF16, tag="wk")
                wv_sb = wkv_p.tile([P, NKT * DH], BF16, tag="wv")
                ropeA_sb = tab_p.tile([P, SEQ], F32R, tag="ra")
                ropeB_sb = tab_p.tile([P, SEQ], F32R, tag="rb")
                masks_sb = tab_p.tile([P, P], BF16, tag="mk")

                def rope_evac(ps_tile, dst_ap, sc, uid):
                    """dst = RoPE(ps_tile), DVE-direct from psum (cross-
                    partition reads are legal when one operand is PSUM)."""
                    cols = slice(sc * SC, (sc + 1) * SC)
                    swp = tmp_p.tile([P, SC], F32R, tag="ropeswp",
                                     name=f"swp{uid}")
                    nc.vector.tensor_mul(swp[0:64, :], ps_tile[64:128, :],
                                         ropeB_sb[0:64, cols])
                    nc.vector.tensor_mul(swp[64:128, :], ps_tile[0:64, :],
                                         ropeB_sb[64:128, cols])
                    nc.vector.tensor_mul(ps_tile[:], ps_tile[:],
                                         ropeA_sb[:, cols])
                    nc.vector.tensor_add(dst_ap, ps_tile[:], swp[:])

                evac = rope_evac


                # 3D views for batched k-tile DMAs: [p, ktile, width]
                xT3 = xT.rearrange("(t p) m -> p t m", p=P)
                wq3 = wq.rearrange("(t p) m -> p t m", p=P)
                wk3 = wk.rearrange("(t p) m -> p t m", p=P)
                wv3 = wv.rearrange("(t p) m -> p t m", p=P)
                wq_sb3 = wq_sb[:].rearrange("p (t m) -> p t m", m=DQ)
                KB = 4  # k-tiles per DMA batch

                def load_xt4(sc, kb):
                    xt4 = xt_p.tile([P, KB * SC], BF16, tag="xt",
                                    name=f"xt{sc}_{kb}")
                    nc.sync.dma_start(
                        xt4[:].rearrange("p (t m) -> p t m", m=SC),
                        xT3[:, kb * KB:(kb + 1) * KB,
                            sc * SC:(sc + 1) * SC])
                    return xt4

                state = {"pending": None}
                qTcs = {}

                def finalize(h, sc, ao, dsum):
                    """normalize head h's attn_outT by 1/denominator.
                    dsum [P, SC] f32 holds the per-q denominator broadcast to
                    all partitions (Pool partition_all_reduce output)."""
                    rec = tmp_p.tile([P, SC], F32, tag="ropest8",
                                     name=f"rec{sc}_{h}")
                    nc.vector.reciprocal_approx_fast(rec[:], dsum[:])
                    nc.vector.tensor_mul(
                        aoT_sb[:, h * SEQ + sc * SC:h * SEQ + (sc + 1) * SC],
                        ao[:], rec[:])

                state["finalize"] = finalize

                def B_heads(sc, heads, qTc, psA=None, mid=None):
                    nkt = 4 * sc + 4

                    # per-kt live column range: diagonal tiles with
                    # alignment a have columns [0, 128a) fully masked —
                    # skip them in scores/exp/PV entirely
                    def lo_of(kt):
                        return 128 * (kt - 4 * sc) if kt >= 4 * sc else 0

                    for h in heads:
                        ao = pacc.tile([P, SC], F32, tag="ao",
                                       name=f"ao{sc}_{h}")
                        # P_all holds all nkt exp'd score tiles; the softmax
                        # denominator is a DVE pair-add tree over them plus
                        # one Pool cross-partition all-reduce — no PE pass
                        P_all = pall_p.tile([P, nkt * SC], BF16, tag="P",
                                            name=f"Pall{sc}_{h}")
                        scr = scr_p.tile([P, (nkt // 2) * SC], BF16,
                                         tag="scr", name=f"scr{sc}_{h}")
                        dsum = dsum_p.tile([P, SC], F32, tag="dsum",
                                           name=f"dsum{sc}_{h}")
                        # zero the dead [0, lo) slices of diagonal tiles so
                        # full-width tree adds see exact zeros there
                        for kt in range(nkt):
                            lo = lo_of(kt)
                            if lo > 0:
                                nc.gpsimd.memset(
                                    P_all[:, kt * SC:kt * SC + lo], 0.0)
                        # producer pass (scores + exp) first, consumers after:
                        # the P_all pool depth is the software-pipeline window
                        for kt in range(nkt):
                            lo = lo_of(kt)
                            S = sB.tile([P, SC], F32, tag="S",
                                        name=f"S{sc}_{h}_{kt}")
                            nc.tensor.matmul(
                                S[:, lo:], kT_sb[:, kt * P:(kt + 1) * P],
                                qTc[:, h * SC + lo:(h + 1) * SC],
                                start=True, stop=True)
                            nc.scalar.activation(
                                P_all[:, kt * SC + lo:(kt + 1) * SC],
                                S[:, lo:],
                                mybir.ActivationFunctionType.Exp)
                            if kt >= 4 * sc:
                                # 0/1 tril on the 128-wide diagonal block,
                                # applied post-exp on the idle Pool engine so
                                # the exp never waits on the DVE queue
                                nc.gpsimd.tensor_mul(
                                    P_all[:, kt * SC + lo:kt * SC + lo + P],
                                    P_all[:, kt * SC + lo:kt * SC + lo + P],
                                    masks_sb[:])
                            if kt % 2 == 1:
                                # first half of the pair-sums go to the Pool
                                # engine (idle), the rest stay on DVE — keeps
                                # each engine under the B-window PE budget
                                eng = (nc.gpsimd if kt < nkt // 2
                                       else nc.vector)
                                with nc.allow_low_precision(
                                        reason="softmax denom pair-sum"):
                                    eng.tensor_add(
                                        scr[:, (kt // 2) * SC:
                                            (kt // 2 + 1) * SC],
                                        P_all[:, (kt - 1) * SC:kt * SC],
                                        P_all[:, kt * SC:(kt + 1) * SC])
                            if kt == 1 and state["pending"] is not None:
                                # previous head's normalization: emitted here
                                # so its DVE chain hides under this head's
                                # producer matmuls
                                finalize(*state["pending"])
                                state["pending"] = None
                        # fold the pair sums, then cross-partition reduce on
                        # the (otherwise idle) Pool engine
                        m = nkt // 2
                        while m > 1:
                            half = (m + 1) // 2
                            with nc.allow_low_precision(
                                    reason="softmax denom tree"):
                                for i in range(m - half):
                                    nc.vector.tensor_add(
                                        scr[:, i * SC:(i + 1) * SC],
                                        scr[:, i * SC:(i + 1) * SC],
                                        scr[:, (i + half) * SC:
                                            (i + half + 1) * SC])
                            m = half
                        nc.gpsimd.partition_all_reduce(
                            dsum[:], scr[:, 0:SC], P, bass_isa.ReduceOp.add)
                        if psA is not None and h + 1 < NH:
                            # next head's RoPE evac: hidden under this head's
                            # consumer matmuls; all-DVE form since ACT (exp)
                            # is the pacing engine in the B windows
                            evac(
                                psA[h + 1],
                                qTc[:, (h + 1) * SC:(h + 2) * SC], sc,
                                f"{sc}_{h + 1}")
                        if state["pending"] is not None:
                            finalize(*state["pending"])
                            state["pending"] = None
                        if mid is not None:
                            mid()
                            mid = None
                        for kt in range(nkt):
                            lo = lo_of(kt)
                            nc.tensor.matmul(
                                ao[:, lo:], v_sb[:, kt * P:(kt + 1) * P],
                                P_all[:, kt * SC + lo:(kt + 1) * SC],
                                start=(kt == 0), stop=(kt == nkt - 1))
                        state["pending"] = (h, sc, ao, dsum)
                    if heads[-1] == NH - 1:
                        finalize(*state["pending"])
                        state["pending"] = None

                def B_gen(sc, qTc):
                    """Generator form of B_heads for weaving into the next
                    chunk's projection stream: S psums come from the 2-bank
                    sB pool, all RoPE evacs live in the A code, and the
                    generator yields at ~1-2us PE granularity so projection
                    matmuls fill every latency chain."""
                    nkt = 4 * sc + 4

                    def lo_of(kt):
                        return 128 * (kt - 4 * sc) if kt >= 4 * sc else 0

                    for h in range(NH):
                        ao = pacc.tile([P, SC], F32, tag="ao",
                                       name=f"ao{sc}_{h}")
                        P_all = pall_p.tile([P, nkt * SC], BF16, tag="P",
                                            name=f"Pall{sc}_{h}")
                        scr = scr_p.tile([P, (nkt // 2) * SC], BF16,
                                         tag="scr", name=f"scr{sc}_{h}")
                        dsum = dsum_p.tile([P, SC], F32, tag="dsum",
                                           name=f"dsum{sc}_{h}")
                        for kt in range(nkt):
                            lo = lo_of(kt)
                            if lo > 0:
                                nc.gpsimd.memset(
                                    P_all[:, kt * SC:kt * SC + lo], 0.0)
                        for kt in range(nkt):
                            lo = lo_of(kt)
                            S = sB.tile([P, SC], F32, tag="S",
                                        name=f"S{sc}_{h}_{kt}")
                            nc.tensor.matmul(
                                S[:, lo:], kT_sb[:, kt * P:(kt + 1) * P],
                                qTc[:, h * SC + lo:(h + 1) * SC],
                                start=True, stop=True)
                            nc.scalar.activation(
                                P_all[:, kt * SC + lo:(kt + 1) * SC],
                                S[:, lo:],
                                mybir.ActivationFunctionType.Exp)
                            if kt >= 4 * sc:
                                nc.gpsimd.tensor_mul(
                                    P_all[:, kt * SC + lo:kt * SC + lo + P],
                                    P_all[:, kt * SC + lo:kt * SC + lo + P],
                                    masks_sb[:])
                            if kt % 2 == 1:
                                eng = (nc.gpsimd if kt < nkt // 2
                                       else nc.vector)
                                with nc.allow_low_precision(
                                        reason="softmax denom pair-sum"):
                                    eng.tensor_add(
                                        scr[:, (kt // 2) * SC:
                                            (kt // 2 + 1) * SC],
                                        P_all[:, (kt - 1) * SC:kt * SC],
                                        P_all[:, kt * SC:(kt + 1) * SC])
                            if kt == 1 and state["pending"] is not None:
                                finalize(*state["pending"])
                                state["pending"] = None
                            if kt % 2 == 1:
                                yield
                        m = nkt // 2
                        while m > 1:
                            half = (m + 1) // 2
                            with nc.allow_low_precision(
                                    reason="softmax denom tree"):
                                for i in range(m - half):
                                    nc.vector.tensor_add(
                                        scr[:, i * SC:(i + 1) * SC],
                                        scr[:, i * SC:(i + 1) * SC],
                                        scr[:, (i + half) * SC:
                                            (i + half + 1) * SC])
                            m = half
                        nc.gpsimd.partition_all_reduce(
                            dsum[:], scr[:, 0:SC], P, bass_isa.ReduceOp.add)
                        if state["pending"] is not None:
                            finalize(*state["pending"])
                            state["pending"] = None
                        for kt in range(nkt):
                            lo = lo_of(kt)
                            nc.tensor.matmul(
                                ao[:, lo:], v_sb[:, kt * P:(kt + 1) * P],
                                P_all[:, kt * SC + lo:(kt + 1) * SC],
                                start=(kt == 0), stop=(kt == nkt - 1))
                            if kt % 4 == 3:
                                yield
                        state["pending"] = (h, sc, ao, dsum)

                def drain(gen, n):
                    for _ in range(n):
                        if next(gen, "done") == "done":
                            return True
                    return False

                # ---------- Phase A(0): single-pass (no weave) -------------
                # q psums from ps6 (4 slots); k/v take the two pacc banks,
                # which carry B-head accumulators only from B(0) onward
                sc = 0
                psA = [ps6.tile([P, SC], F32, tag="ps6", name=f"psA0_{j}")
                       for j in range(4)]
                psA.append(pacc.tile([P, SC], F32, tag="ao", name="psA0_4"))
                psA.append(pacc.tile([P, SC], F32, tag="ao", name="psA0_5"))
                # heartbeat: a dependency-free 1-column matmul that the
                # engine executes ~0.7us after the barrier, so the idle
                # gap to the first data-fed matmul (~3.6us) stays under
                # the p-state ramp re-arm threshold (~3.2us of engine
                # idleness); its start+stop group in psA[0] fully
                # precedes the real accumulation group (sequential
                # groups in one bank are safe; interleaved are not)
                hb = tab_p.tile([P, 8], BF16, tag="hb")
                nc.gpsimd.memset(hb[:], 0.0)
                nc.tensor.matmul(psA[0][0:8, 0:1], hb[:], hb[:, 0:1],
                                 start=True, stop=True)
                boot_sb = boot_p.tile([P, 8 * 2 * SC], BF16)
                boot3 = boot_sb[:].rearrange("p (t m) -> p t m", m=2 * SC)
                bootd3 = boot.rearrange("p (t m) -> p t m", m=2 * SC)

                def boot_wq(k, j):
                    return boot_sb[:, k * 2 * SC + j * DH:
                                   k * 2 * SC + (j + 1) * DH]

                def boot_x(k):
                    return boot_sb[:, k * 2 * SC + SC:(k + 1) * 2 * SC]

                for kb in range(NKT // KB):
                    # kb0/kb1 stream as per-ktile (wq|x) pairs from the boot
                    # tensor, alternating the SP and ACT issue queues so the
                    # 650ns-per-DMA single-queue serialization never gates
                    # the feed; kb2+ use the batched tensors (the bus has
                    # built ~1us of slack by then)
                    ksl = slice(kb * KB, (kb + 1) * KB)
                    if kb < 2:
                        # per-ktile (wq_k|x_k) 256KB pair pieces on the SP
                        # queue: per-piece sems track the bus exactly
                        # (need(k) - ready(k) = 728+124k > 0, so the feed
                        # never stalls after the first matmul at ~3.6us).
                        # The heartbeat matmul above keeps the PE engine's
                        # idle gap under the ~3.2us ramp re-arm threshold
                        # that per-pair sems would otherwise trip (pair
                        # sems release the blocked sequencer exactly at
                        # engine-start; without the heartbeat that first
                        # engine event lands at 3.6us idle -> every
                        # dispatch in the next 3us runs at half clock,
                        # +5.7us total).
                        xt4 = None
                        for k in range(kb * KB, (kb + 1) * KB):
                            nc.sync.dma_start(
                                boot_sb[:, k * 2 * SC:(k + 1) * 2 * SC],
                                boot[:, k * 2 * SC:(k + 1) * 2 * SC])
                    else:
                        xh = xt_p.tile([P, KB * SC], BF16, tag="xt",
                                       name=f"xt0_{kb}")
                        xhv = xh[:].rearrange("p (t m) -> p t m", m=SC)
                        nc.sync.dma_start(wq_sb3[:, ksl, :],
                                          wq3[:, ksl, :])
                        nc.sync.dma_start(xhv[:, 0:2, :],
                                          xT3[:, kb * KB:kb * KB + 2,
                                              0:SC])
                        nc.sync.dma_start(xhv[:, 2:KB, :],
                                          xT3[:, kb * KB + 2:(kb + 1) * KB,
                                              0:SC])
                        xt4 = xh
                    # k/v weights trail by one batch; kb0's tiles are
                    # accumulated at the end with the start flag on k-tile 4
                    if kb > 0:
                        nc.sync.dma_start(
                            wk_sb[:].rearrange("p (t m) -> p t m",
                                               m=DH)[:, ksl, :],
                            wk3[:, ksl, :])
                        nc.sync.dma_start(
                            wv_sb[:].rearrange("p (t m) -> p t m",
                                               m=DH)[:, ksl, :],
                            wv3[:, ksl, :])
                    if kb == 5:
                        # kb0's k/v weights: only consumed by the deferred
                        # tail after kb7, so they stream late
                        nc.sync.dma_start(
                            wk_sb[:].rearrange("p (t m) -> p t m",
                                               m=DH)[:, 0:KB, :],
                            wk3[:, 0:KB, :])
                        nc.sync.dma_start(
                            wv_sb[:].rearrange("p (t m) -> p t m",
                                               m=DH)[:, 0:KB, :],
                            wv3[:, 0:KB, :])
                    def mm_at0(j, ki, xt4=None, kb=None):
                        k = kb * KB + ki
                        if xt4 is not None:
                            xt = xt4[:, ki * SC:(ki + 1) * SC]
                        else:
                            xt = boot_x(k)
                        if j < NH:
                            st, sp = (k == 0), (k == NKT - 1)
                            if k < 2 * KB:
                                w_ap = boot_wq(k, j)
                            else:
                                w_ap = wq_sb[:, k * DQ + j * DH:
                                             k * DQ + (j + 1) * DH]
                        else:
                            st, sp = (k == KB), (k == KB - 1)
                            w_ap = (wk_sb if j == 4 else
                                    wv_sb)[:, k * DH:(k + 1) * DH]
                        nc.tensor.matmul(psA[j][:], w_ap, xt,
                                         start=st, stop=sp)

                    if kb == 0:
                        # ki-major matches the per-piece feed granularity
                        for ki in range(KB):
                            for j in range(4):
                                mm_at0(j, ki, xt4, kb)
                    elif kb == 1:
                        # q heads ki-major first, then this batch's k/v in
                        # a trailing pass (their wk/wv DMAs land meanwhile)
                        for ki in range(KB):
                            for j in range(4):
                                mm_at0(j, ki, xt4, kb)
                        for j in (4, 5):
                            for ki in range(KB):
                                mm_at0(j, ki, xt4, kb)
                    else:
                        # j-major, q first: the k/v matmuls only run after
                        # 16 q matmuls of cover, so trailing wk/wv DMA
                        # batches never stall the PE during the ramp
                        for j in (0, 1, 2, 3, 4, 5):
                            for ki in range(KB):
                                mm_at0(j, ki, xt4, kb)
                    if kb == 4:
                        # chunk-0's slice of the rope tables plus consts:
                        # 0.6MB fits the DMA slack of one k-batch; the rest
                        # streams during A(1)
                        nc.sync.dma_start(ropeA_sb[:, 0:SC], ropeA[:, 0:SC])
                        nc.sync.dma_start(ropeB_sb[:, 0:SC], ropeB[:, 0:SC])
                        nc.sync.dma_start(masks_sb[:], masks[:])
                        nc.sync.dma_start(ident_sb[:], ident[:])
                # deferred k/v tiles 0..3 (x read back from the live boot
                # tile — no re-fetch)
                for j in (4, 5):
                    for ki in range(KB):
                        mm_at0(j, ki, None, 0)
                qTc = qTc_p.tile([P, NH * SC], BF16, tag="qTc", name="qTc0")
                qTcs[0] = qTc
                evac(psA[0], qTc[:, 0:SC], 0, "0_0")
                evac(psA[4], kT_sb[:, 0:SC], 0, "0_k")
                vtmp = vt_p.tile([P, SC], BF16, tag="vtmp")
                nc.scalar.copy(vtmp[:], psA[5][:])
                for t in range(4):
                    ptr = sB.tile([P, P], BF16, tag="S", name=f"ptr0_{t}")
                    nc.tensor.transpose(ptr[:], vtmp[:, t * P:(t + 1) * P],
                                        ident_sb[:])
                    nc.scalar.copy(v_sb[:, t * P:(t + 1) * P], ptr[:])
                for j in (1, 2, 3):
                    evac(psA[j], qTc[:, j * SC:(j + 1) * SC],
                                     0, f"0_{j}")

                # ---------- Phases A(1..3), each weaving B(sc-1) -----------
                # two segments per chunk: q-heads (4 ps6 banks) then k/v
                # (ps6 slots freed by the q evacs); x streams twice; the
                # previous chunk's B work drains between k-batches so its
                # exp/DVE/Pool chains hide under dense projection matmuls
                for sc in (1, 2, 3):
                    scols = slice(sc * SC, (sc + 1) * SC)
                    gen = B_gen(sc - 1, qTcs[sc - 1])
                    done = False
                    nkt_w = 4 * sc  # prev chunk's nkt
                    # total yields: producer nkt/2 + consumer nkt/4 per head
                    pieces = 4 * (nkt_w // 2 + (nkt_w + 3) // 4)
                    per_kb = max(1, (pieces + 15) // 16)
                    qTc = qTc_p.tile([P, NH * SC], BF16, tag="qTc",
                                     name=f"qTc{sc}")
                    qTcs[sc] = qTc
                    psq = [ps6.tile([P, SC], F32, tag="ps6",
                                    name=f"psq{sc}_{j}") for j in range(4)]

                    def mmq(j, ki, xt4=None, kb=None):
                        k = kb * KB + ki
                        # wq ktiles 0..7 live in the resident boot tile
                        # (A(0)'s startup stream); 8..31 in wq_sb — saves
                        # re-loading 1MB of wq during A(0)
                        w_ap = (boot_wq(k, j) if k < 2 * KB else
                                wq_sb[:, k * DQ + j * DH:
                                      k * DQ + (j + 1) * DH])
                        nc.tensor.matmul(
                            psq[j][:], w_ap,
                            xt4[:, ki * SC:(ki + 1) * SC],
                            start=(k == 0), stop=(k == NKT - 1))

                    for kb in range(NKT // KB):
                        xt4 = load_xt4(sc, kb)
                        if kb == 3:
                            # this chunk's slice of the rope tables, just in
                            # time for the evacs: 0.5MB rides one k-batch's
                            # DMA slack without stalling the x stream
                            nc.sync.dma_start(ropeA_sb[:, scols],
                                              ropeA[:, scols])
                            nc.sync.dma_start(ropeB_sb[:, scols],
                                              ropeB[:, scols])
                        if kb == NKT // KB - 1:
                            for j in range(4):      # q0 completes first
                                for ki in range(KB):
                                    mmq(j, ki, xt4, kb)
                        else:
                            for ki in range(KB):
                                for j in range(4):
                                    mmq(j, ki, xt4, kb)
                        done = drain(gen, per_kb) or done
                    evac(psq[0], qTc[:, 0:SC], sc, f"{sc}_0")
                    evac(psq[1], qTc[:, SC:2 * SC], sc, f"{sc}_1")
                    # k/v segment: psums from the ps6 slots the q evacs free
                    psk = ps6.tile([P, SC], F32, tag="ps6", name=f"psk{sc}")
                    psv = ps6.tile([P, SC], F32, tag="ps6", name=f"psv{sc}")

                    def mmkv(ps, w_sb, ki, xt4=None, kb=None):
                        k = kb * KB + ki
                        nc.tensor.matmul(
                            ps[:], w_sb[:, k * DH:(k + 1) * DH],
                            xt4[:, ki * SC:(ki + 1) * SC],
                            start=(k == 0), stop=(k == NKT - 1))

                    for kb in range(NKT // KB):
                        xt4 = load_xt4(sc, kb)
                        if kb == NKT // KB - 1:
                            for ki in range(KB):    # k completes first
                                mmkv(psk, wk_sb, ki, xt4, kb)
                            for ki in range(KB):
                                mmkv(psv, wv_sb, ki, xt4, kb)
                        else:
                            for ki in range(KB):
                                mmkv(psk, wk_sb, ki, xt4, kb)
                                mmkv(psv, wv_sb, ki, xt4, kb)
                        if kb == 0:
                            evac(psq[2], qTc[:, 2 * SC:3 * SC],
                                             sc, f"{sc}_2")
                        if kb == 1:
                            evac(psq[3], qTc[:, 3 * SC:4 * SC],
                                             sc, f"{sc}_3")
                        done = drain(gen, per_kb) or done
                    evac(psk, kT_sb[:, scols], sc, f"{sc}_k")
                    vtmp = vt_p.tile([P, SC], BF16, tag="vtmp")
                    nc.scalar.copy(vtmp[:], psv[:])
                    for t in range(4):
                        ptr = sB.tile([P, P], BF16, tag="S",
                                      name=f"ptr{sc}_{t}")
                        nc.tensor.transpose(ptr[:], vtmp[:, t * P:(t + 1) * P],
                                            ident_sb[:])
                        nc.scalar.copy(
                            v_sb[:, (sc * 4 + t) * P:(sc * 4 + t + 1) * P],
                            ptr[:])
                    while not done:
                        done = drain(gen, 4)

                # B(3) head 0 is emitted before the weight-pool release so
                # its producer runs under the release barrier + wo DMA
                B_heads(NSC - 1, (0,), qTcs[NSC - 1])

                # free the projection weights/x pools before phase C so wo
                # can be resident while B(3) runs
                inner.close()

                # ------ Phase B(3) woven with phase C ----------------------
                # C s-tiles 0..11 depend only on B(0..2); interleave them
                # with B(3)'s heads to fill its latency chains
                with tc.tile_pool(name="wo_p", bufs=1) as wo_p, \
                     tc.tile_pool(name="out_p", bufs=3) as out_p:
                    wo_t = wo_p.tile([P, 4 * DIM], BF16, tag="wo")
                    # load wo by 512-wide column block (all 4 head-rows per
                    # DMA) so C's first matmuls start after 0.5MB, not 4MB
                    wo3d = wo.rearrange("(t p) m -> p t m", p=P)
                    wo_t3 = wo_t[:].rearrange("p (t m) -> p t m", m=DIM)
                    for dc in range(8):
                        nc.sync.dma_start(
                            wo_t3[:, :, dc * SC:(dc + 1) * SC],
                            wo3d[:, :, dc * SC:(dc + 1) * SC])

                    def C_st(sts, last_st=None):
                        for st in sts:
                            ot = out_p.tile([P, DIM], BF16, tag="ot",
                                            name=f"ot{st}")
                            last = st == last_st
                            def po_group(cl, cw, psname):
                                """psum group for out cols [cl, cl+cw) of
                                this s-tile; returns the stopped psum."""
                                po = ps6.tile([P, cw], F32, tag="ps6",
                                              name=psname)
                                for h in range(NH):
                                    nc.tensor.matmul(
                                        po[:],
                                        aoT_sb[:, h * SEQ + st * P:
                                               h * SEQ + (st + 1) * P],
                                        wo_t[:, h * DIM + cl:
                                             h * DIM + cl + cw],
                                        start=(h == 0), stop=(h == NH - 1))
                                return po

                            if last:
                                # tail-optimized order: the final chain is
                                # small-evac + one hwdge DMA (625+650+tr+900).
                                # Queue discipline: the SP queue must be free
                                # when the last piece's data lands, and the
                                # ACT queue must carry no DMA that precedes
                                # the last evac in program order, so the
                                # trailing "six" DMA rides ACT, everything
                                # else SP, and all late evacs go to DVE.
                                pieces = [(dc * SC, SC, None)
                                          for dc in range(7)]
                                pieces += [(7 * SC, 192, "y"),
                                           (7 * SC + 192, 320, "z")]
                                otY = out_p.tile([P, 192], BF16,
                                                 tag="otY", name="otY")
                                for pi, (cl, cw, nm) in enumerate(pieces):
                                    po = po_group(cl, cw,
                                                  f"po{st}_{nm or pi}")
                                    if nm == "y":
                                        # y evacs on ACT into its own tile
                                        # (same-tile cross-engine writes get
                                        # WAW-serialized by the framework)
                                        # and stores via the Pool SWDGE path
                                        # — off both SP.SEQ and HWDGE
                                        nc.scalar.copy(otY[:], po[:])
                                        nc.gpsimd.dma_start(
                                            out[st * P:(st + 1) * P,
                                                cl:cl + cw], otY[:])
                                        continue
                                    if nm == "z":
                                        # the true tail: DVE evac (engine is
                                        # clear) + SP store; chain is
                                        # evac+25+625+650+tr+900 from T_pe
                                        nc.vector.tensor_copy(
                                            ot[:, cl:cl + cw], po[:])
                                        nc.sync.dma_start(
                                            out[st * P:(st + 1) * P,
                                                cl:cl + cw],
                                            ot[:, cl:cl + cw])
                                        continue
                                    eng = (nc.scalar if pi % 4 == 3
                                           else nc.vector)
                                    (eng.copy if eng is nc.scalar
                                     else eng.tensor_copy)(
                                        ot[:, cl:cl + cw], po[:])
                                    nc.sync.dma_start(
                                        out[st * P:(st + 1) * P,
                                            cl:cl + cw],
                                        ot[:, cl:cl + cw])
                                continue
                            for dc in range(8):
                                po = po_group(dc * SC, SC, f"po{st}_{dc}")
                                # po evacuation mostly on DVE: ACT carries
                                # the exps in this phase; it takes every
                                # fourth copy to keep DVE off the po rotation
                                # critical path
                                if dc % 4 == 3:
                                    nc.scalar.copy(
                                        ot[:, dc * SC:(dc + 1) * SC], po[:])
                                else:
                                    nc.vector.tensor_copy(
                                        ot[:, dc * SC:(dc + 1) * SC], po[:])
                                if last:
                                    # drain the final s-tile in eighths so
                                    # the kernel tail is one short DMA
                                    nc.sync.dma_start(
                                        out[st * P:(st + 1) * P,
                                            dc * SC:(dc + 1) * SC],
                                        ot[:, dc * SC:(dc + 1) * SC])
                            if not last:
                                nc.sync.dma_start(
                                    out[st * P:(st + 1) * P, :], ot[:])

                    # weave the deferred B(0) and B(3) heads 1-3 between C
                    # s-tile groups so every latency chain has dense PE cover;
                    # C group g needs B(g) complete, so C(0..3) follows B(0)
                    # and C(12..15) follows B(3)h3 (which gets C(11) as its
                    # mid-head cover)
                    q3 = qTcs[NSC - 1]
                    C_st([0])
                    B_heads(NSC - 1, (1,), q3, mid=lambda: C_st([1]))
                    C_st([2, 3, 4])
                    B_heads(NSC - 1, (2,), q3, mid=lambda: C_st([5]))
                    C_st([6, 7, 8, 9])
                    B_heads(NSC - 1, (3,), q3, mid=lambda: C_st([10]))
                    C_st([11, 12, 13, 14, 15], last_st=15)
    nc.compile()
    return nc


def make_in_maps(x, freqs_cos, freqs_sin, wq, wk, wv, wo):
    """Host-side sharding + layout prep. Returns list of 8 per-core dicts."""
    import ml_dtypes
    bf16 = np.dtype(ml_dtypes.bfloat16)
    f32 = np.float32
    x2 = np.asarray(x, f32).reshape(SEQ, DIM)
    xT = np.ascontiguousarray(x2.T).astype(bf16)
    # RoPE de-interleave permutation within each head: evens then odds
    perm = np.concatenate([np.arange(0, DH, 2), np.arange(1, DH, 2)])
    scale = 1.0 / np.sqrt(np.float32(DH))
    cosT = np.ascontiguousarray(np.asarray(freqs_cos, f32).T)   # [64, SEQ]
    sinT = np.ascontiguousarray(np.asarray(freqs_sin, f32).T)
    ropeA = np.concatenate([cosT, cosT], axis=0)                # [128, SEQ]
    ropeB = np.concatenate([-sinT, sinT], axis=0)
    # multiplicative causal tril for the 128-wide diagonal block:
    # keep (1) where qq - kk >= 0 else drop (0); applied to exp'd scores
    kk = np.arange(P)[:, None]
    qq = np.arange(P)[None, :]
    masks = np.where(qq - kk >= 0, 1.0, 0.0).astype(bf16)
    ident = np.eye(P, dtype=bf16)

    wq_f = np.asarray(wq, f32)
    wk_f = np.asarray(wk, f32)
    wv_f = np.asarray(wv, f32)
    wo_f = np.asarray(wo, f32)
    in_maps = []
    for c in range(NCORES):
        wq_c = wq_f[:, c * DQ:(c + 1) * DQ].reshape(DIM, NH, DH)[:, :, perm]
        wq_c = np.ascontiguousarray(wq_c.reshape(DIM, DQ) * scale).astype(bf16)
        wk_c = np.ascontiguousarray(
            wk_f[:, c * DH:(c + 1) * DH][:, perm]).astype(bf16)
        wv_c = np.ascontiguousarray(
            wv_f[:, c * DH:(c + 1) * DH]).astype(bf16)
        wo_c = np.ascontiguousarray(wo_f[c * DQ:(c + 1) * DQ, :]).astype(bf16)
        # startup stream: (wq k-tile | x chunk-0 k-tile) pairs for k=0..7
        boot = np.empty((P, 8 * 2 * SC), bf16)
        for k in range(8):
            boot[:, k * 2 * SC:k * 2 * SC + SC] = wq_c[k * P:(k + 1) * P, :]
            boot[:, k * 2 * SC + SC:(k + 1) * 2 * SC] = \
                xT[k * P:(k + 1) * P, 0:SC]
        in_maps.append({
            "xT": xT, "wq": wq_c, "wk": wk_c, "wv": wv_c, "wo": wo_c,
            "boot": np.ascontiguousarray(boot),
            "ropeA": ropeA, "ropeB": ropeB, "masks": masks, "ident": ident,
        })
    return in_maps


_NC_CACHE = None


def kernel(x, freqs_cos, freqs_sin, mask, wq, wk, wv, wo):
    """Full-input entry point: returns [1, 2048, 4096] float32."""
    global _NC_CACHE
    from concourse.bass_utils import run_bass_kernel_spmd
    if _NC_CACHE is None:
        _NC_CACHE = build_nc()
    in_maps = make_in_maps(x, freqs_cos, freqs_sin, wq, wk, wv, wo)
    res = run_bass_kernel_spmd(_NC_CACHE, in_maps, core_ids=list(range(NCORES)))
    acc = np.zeros((SEQ, DIM), np.float32)
    for c in range(NCORES):
        acc += res.results[c]["out"].astype(np.float32)
    return acc.reshape(BS, SEQ, DIM)

